# revision 2
# baseline (speedup 1.0000x reference)
"""BiLSTM Enc-Dec + CRF NLL loss on 2 Trainium2 cores (SPMD, fwd/bwd split).

Strategy
--------
Batch=1 sequence, T=2048. The four BiLSTM scans (enc L0 -> enc L1 -> dec L0
-> dec L1) are inherently sequential in time; within each layer the forward
and backward direction are independent. Core 0 runs all forward-direction
scans, core 1 runs all backward-direction scans, with one identical
(symmetric) SPMD program. Direction asymmetry is absorbed into per-core
*data*: core 1 receives the embedding sequence time-reversed so its
"forward" scan IS the backward scan; cross-core exchanges (layer outputs,
final states) use AllGather on internal DRAM bounce buffers.

Input projections x @ W_ih^T for a whole layer are big parallel matmuls
computed once per stage into DRAM (fp32), streamed into SBUF in windows
during the scan. The recurrent matvec h @ W_hh^T runs on the tensor engine
as 64 [128x128] fp8e4m3 weight-stationary matmuls per step (fp8 halves the
LDWEIGHTS time vs bf16 under FWL, which dominates at moving-dim 1). Gates
stay in the natural PyTorch order [i, f, g, o]; the i/f/g gate tiles issue
first so the cell-state update chain overlaps the o-gate matmuls. h is kept
in a small statically-addressed rotating buffer (hbuf) so the matmul moving
operands have compile-time addresses; one vector copy per unrolled loop
iteration spills hbuf into the full history Hs (fp8), which later feeds the
next layer's input projection, the cross-core exchange and the feats matmul.

The CRF forward pass runs in the linear domain: alpha' = exp(trans) @ alpha
(a single stationary 48x48 matmul per step) times exp(feats_t), renormalized
each step by its sum; log of the normalizer is accumulated on the host in
float64. The CRF score term is computed on the host from device feats.
"""

import sys

sys.path.insert(0, "/opt/trn_rl_repo")

import numpy as np
import ml_dtypes

import concourse.bacc as bacc
import concourse.mybir as mybir
from concourse.bass import ds
from concourse.tile import TileContext
from concourse.bass_utils import run_bass_kernel_spmd

# problem dims (hardcoded per spec)
T = 2048
ELMO = 1024
H = 512
POS = 64
K = 48
S = 50
L = 2
NEG = -10000.0
START_IDX, END_IDX = 0, 1

Din0 = ELMO + POS  # 1088
K0C = 9  # ceil(1088/128) k-tiles for layer-0 input (padded to 1152)
HC = 4  # h chunks of 128
G = 4 * H  # 2048 gates
GC = 16  # gate chunks of 128
U = 16  # scan steps unrolled per hardware-loop iteration
CH = 256  # scan steps per xp SBUF window
UCRF = 16

bf16 = mybir.dt.bfloat16
f8 = mybir.dt.float8e4
f32 = mybir.dt.float32
AF = mybir.ActivationFunctionType
ALU = mybir.AluOpType
ET = mybir.EngineType

np_f8 = ml_dtypes.float8_e4m3
np_bf16 = ml_dtypes.bfloat16

_CACHE = {}


# ----------------------------------------------------------------------------
# host-side weight preparation
# ----------------------------------------------------------------------------

def _tile_kT(wT, nk):
    """[Ktot, M] -> [128, nk*M] with col kc*M + m = wT[kc*128 + p, m]."""
    Ktot, M = wT.shape
    assert Ktot == nk * 128
    return np.ascontiguousarray(wT.reshape(nk, 128, M).transpose(1, 0, 2).reshape(128, nk * M))


def _prep_core(inputs, d):
    """Build the per-core input map for direction d (0=fwd core, 1=bwd core)."""
    f = np.float32
    ins = {}
    sentence = inputs["sentence"].astype(f)
    pos_emb = inputs["pos_emb"].astype(f)
    speech = inputs["speech_tags"].astype(np.int64)
    embeds = np.concatenate([sentence, pos_emb[speech]], axis=1)  # (T, 1088)
    if d == 1:
        embeds = embeds[::-1]
    embT = np.zeros((K0C * 128, T), f)
    embT[:Din0] = embeds.T
    ins["embT"] = _tile_kT(embT, K0C).astype(np_bf16)

    for model in ("enc", "dec"):
        for layer in (0, 1):
            whh = inputs[f"{model}_w_hh{layer}"][d].astype(f)  # (2048, 512)
            ins[f"whhT_{model}{layer}"] = _tile_kT(
                np.ascontiguousarray(whh.T), HC
            ).astype(np_f8)
            b = (inputs[f"{model}_b_ih{layer}"][d] + inputs[f"{model}_b_hh{layer}"][d]).astype(f)
            ins[f"bias_{model}{layer}"] = np.ascontiguousarray(
                b.reshape(GC, 128).T
            ).astype(f)  # [128,16] col mc
        wih0 = inputs[f"{model}_w_ih0"][d].astype(f)  # (2048, 1088)
        w0T = np.zeros((K0C * 128, G), f)
        w0T[:Din0] = wih0.T
        ins[f"wih0T_{model}"] = _tile_kT(w0T, K0C).astype(np_bf16)
        wih1 = inputs[f"{model}_w_ih1"][d].astype(f)  # (2048, 1024)
        own = wih1[:, d * H : (d + 1) * H]
        peer = wih1[:, (1 - d) * H : (2 - d) * H]
        ins[f"wih1T_own_{model}"] = _tile_kT(np.ascontiguousarray(own.T), HC).astype(np_f8)
        ins[f"wih1T_peer_{model}"] = _tile_kT(np.ascontiguousarray(peer.T), HC).astype(np_f8)

    # e2h/e2c: rows = own dec init states, cols permuted to AllGather order.
    # AG order of the 2048-dim enc state: [c0_l0, c0_l1, c1_l0, c1_l1]
    # (c0 = fwd dir, c1 = bwd dir); PyTorch flat order is [l0f, l0b, l1f, l1b].
    col_perm = np.concatenate(
        [
            np.arange(0, H),  # l0f
            np.arange(2 * H, 3 * H),  # l1f
            np.arange(H, 2 * H),  # l0b
            np.arange(3 * H, 4 * H),  # l1b
        ]
    )
    # own dec-init rows: init_h.reshape(2L, H)[j] is state for scan order
    # [dl0_f, dl0_b, dl1_f, dl1_b]; core d needs rows for [dl0 dir d, dl1 dir d]
    row_sel = np.concatenate([np.arange(d * H, (d + 1) * H), np.arange((2 + d) * H, (3 + d) * H)])
    for nm in ("e2h", "e2c"):
        w = inputs[f"{nm}_w"].astype(f)[row_sel][:, col_perm]  # (1024, 2048)
        ins[f"{nm}T"] = _tile_kT(np.ascontiguousarray(w.T), GC).astype(np_bf16)
        b = inputs[f"{nm}_b"].astype(f)[row_sel]  # (1024,)
        ins[f"{nm}_b"] = np.ascontiguousarray(b.reshape(8, 128).T).astype(f)  # [128, 8]

    # feats weights: rank0 half multiplies fwd-core outputs, rank1 half the
    # bwd-core outputs (identical on both cores; feats computed redundantly)
    h2t = inputs["h2t_w"].astype(f)
    ins["h2tT_r0"] = _tile_kT(np.ascontiguousarray(h2t[:, 0:H].T), HC).astype(np_f8)
    ins["h2tT_r1"] = _tile_kT(np.ascontiguousarray(h2t[:, H:].T), HC).astype(np_f8)
    ins["h2t_b"] = inputs["h2t_b"].astype(f).reshape(K, 1)

    trans = inputs["transitions"].astype(f)
    ins["transT"] = np.ascontiguousarray(trans.T)
    ins["transEnd"] = np.ascontiguousarray(trans[END_IDX].reshape(K, 1))
    a0 = np.full((K, 1), 0.0, f)
    a0[:, 0] = 0.0
    a0[START_IDX, 0] = 1.0
    ins["alpha0"] = a0
    return ins


# ----------------------------------------------------------------------------
# device program
# ----------------------------------------------------------------------------

def build():
    nc = bacc.Bacc("TRN2", target_bir_lowering=False, num_devices=2)

    def din(name, shape, dt=bf16):
        return nc.dram_tensor(name, shape, dt, kind="ExternalInput")

    embT_d = din("embT", [128, K0C * T])
    whh_d = {s: din(f"whhT_{s}", [128, HC * G], f8) for s in ("enc0", "enc1", "dec0", "dec1")}
    bias_d = {s: din(f"bias_{s}", [128, GC], f32) for s in ("enc0", "enc1", "dec0", "dec1")}
    wih0_d = {m: din(f"wih0T_{m}", [128, K0C * G]) for m in ("enc", "dec")}
    wih1o_d = {m: din(f"wih1T_own_{m}", [128, HC * G], f8) for m in ("enc", "dec")}
    wih1p_d = {m: din(f"wih1T_peer_{m}", [128, HC * G], f8) for m in ("enc", "dec")}
    e2hT_d = din("e2hT", [128, GC * 1024])
    e2cT_d = din("e2cT", [128, GC * 1024])
    e2hb_d = din("e2h_b", [128, 8], f32)
    e2cb_d = din("e2c_b", [128, 8], f32)
    h2tT_r0_d = din("h2tT_r0", [128, HC * K], f8)
    h2tT_r1_d = din("h2tT_r1", [128, HC * K], f8)
    h2tb_d = din("h2t_b", [K, 1], f32)
    transT_d = din("transT", [K, K], f32)
    transEnd_d = din("transEnd", [K, 1], f32)
    alpha0_d = din("alpha0", [K, 1], f32)

    feats_out = nc.dram_tensor("feats", [K, T], f32, kind="ExternalOutput")
    lnS_out = nc.dram_tensor("lnS", [1, T], f32, kind="ExternalOutput")
    zfin_out = nc.dram_tensor("zfin", [1, 1], f32, kind="ExternalOutput")

    # internal DRAM
    xp_a = nc.dram_tensor("xp_a", [128, GC * T], f32)  # enc0 / enc1 / dec1
    xp_b = nc.dram_tensor("xp_b", [128, GC * T], f32)  # dec0
    hs_ag_in = nc.dram_tensor("hs_ag_in", [128, HC * (T + 1)], f8)
    hs_ag_out = nc.dram_tensor("hs_ag_out", [256, HC * (T + 1)], f8)
    fin_ag_in = nc.dram_tensor("fin_ag_in", [128, 16], f32)
    fin_ag_out = nc.dram_tensor("fin_ag_out", [256, 16], f32)

    RG = [[0, 1]]

    with TileContext(nc) as tc:
        with (
            tc.tile_pool(name="pw", bufs=1) as pw,  # persistent weights/state
            tc.tile_pool(name="slab1", bufs=1) as slab1_pool,  # whh / wih1 own
            tc.tile_pool(name="slab2", bufs=1) as slab2_pool,  # wih1 peer
            tc.tile_pool(name="hs", bufs=2) as hs_pool,
            tc.tile_pool(name="peer", bufs=1) as peer_pool,
            tc.tile_pool(name="xpw", bufs=2) as xpw_pool,
            tc.tile_pool(name="psx", bufs=2, space="PSUM") as psx_pool,  # xp matmuls
            tc.tile_pool(name="pss", bufs=4, space="PSUM") as pss_pool,  # scan
            tc.tile_pool(name="psm", bufs=2, space="PSUM") as psm_pool,  # crf
        ):
            # ---- persistent loads
            bias = {}
            for s in ("enc0", "enc1", "dec0", "dec1"):
                bias[s] = pw.tile([128, GC], f32, name=f"bias_{s}")
                nc.sync.dma_start(out=bias[s], in_=bias_d[s][:, :])

            # ---- xp matmul helper: out_dram[:, mc*T + t] over given k-slabs
            def xp_matmul(out_dram, slabs, bias_tile):
                """slabs: list of (sbuf_slab, nk, rhs_fn) triples contracting
                consecutive k-ranges; rhs_fn(kc, t0, n) -> AP [128, n] moving."""
                NT = 512
                for tb in range(T // NT):
                    t0 = tb * NT
                    for mc in range(GC):
                        ps = psx_pool.tile([128, NT], f32, tag="psx", name=f"psx_{tb}_{mc}")
                        first = True
                        for slab, nk, rhs_fn in slabs:
                            for kc in range(nk):
                                nc.tensor.matmul(
                                    ps,
                                    slab[:, kc * G + mc * 128 : kc * G + (mc + 1) * 128],
                                    rhs_fn(kc, t0, NT),
                                    start=first,
                                    stop=(slab is slabs[-1][0]) and kc == nk - 1,
                                )
                                first = False
                        st = xpw_pool.tile([128, NT], f32, tag="xstage", name=f"xst_{tb}_{mc}")
                        nc.vector.tensor_scalar(
                            out=st, in0=ps, scalar1=bias_tile[:, mc : mc + 1],
                            scalar2=None, op0=ALU.add,
                        )
                        nc.sync.dma_start(
                            out=out_dram[:, mc * T + t0 : mc * T + t0 + NT], in_=st
                        )

            # ---- P0: layer-0 xp for enc and dec (embT and wih0 streamed
            # in windows; weight window per (tb, mc): [128, K0C, 128])
            embr = embT_d[:, :].rearrange("p (k t) -> p k t", k=K0C)
            NT = 512
            for model, out_dram in (("enc", xp_a), ("dec", xp_b)):
                w0r = wih0_d[model][:, :].rearrange("p (k m) -> p k m", k=K0C)
                for tb in range(T // NT):
                    t0 = tb * NT
                    ew = xpw_pool.tile([128, K0C, NT], bf16, tag="win", name=f"ew_{model}_{tb}")
                    nc.sync.dma_start(out=ew, in_=embr[:, :, t0 : t0 + NT])
                    for mc in range(GC):
                        ww = xpw_pool.tile(
                            [128, K0C, 128], bf16, tag="wwin", name=f"ww_{model}_{tb}_{mc}"
                        )
                        nc.sync.dma_start(
                            out=ww, in_=w0r[:, :, mc * 128 : (mc + 1) * 128]
                        )
                        ps = psx_pool.tile([128, NT], f32, tag="psx", name=f"psx0_{model}_{tb}_{mc}")
                        for kc in range(K0C):
                            nc.tensor.matmul(
                                ps, ww[:, kc, :], ew[:, kc, :],
                                start=(kc == 0), stop=(kc == K0C - 1),
                            )
                        st = xpw_pool.tile([128, NT], f32, tag="xstage", name=f"x0_{model}_{tb}_{mc}")
                        nc.vector.tensor_scalar(
                            out=st, in0=ps, scalar1=bias[f"{model}0"][:, mc : mc + 1],
                            scalar2=None, op0=ALU.add,
                        )
                        nc.sync.dma_start(
                            out=out_dram[:, mc * T + t0 : mc * T + t0 + NT], in_=st
                        )

            # ---- scan helper
            # gates in natural PyTorch order [i, f, g, o]; i/f/g tiles (mc
            # 0..11) issue first so the c-update overlaps the o-gate matmuls.
            def scan(s, xp_dram, Hs, c, h0_src=None, c0_src=None):
                """Run one LSTM direction scan. Hs: [128, HC*(T+1)] f8 tile;
                c: [128, HC] f32 tile. h0/c0 default zero."""
                W = slab1_pool.tile([128, HC * G], f8, tag="whh", name=f"whh_{s}")
                nc.sync.dma_start(out=W, in_=whh_d[s][:, :])
                hbuf = pw.tile([128, U * HC], f8, tag="hbuf", name=f"hbuf_{s}")
                if h0_src is None:
                    nc.vector.memset(Hs[:, 0:HC], 0.0)
                    nc.vector.memset(hbuf[:, (U - 1) * HC : U * HC], 0.0)
                    nc.vector.memset(c, 0.0)
                else:
                    nc.vector.tensor_copy(Hs[:, 0:HC], h0_src)
                    nc.vector.tensor_copy(hbuf[:, (U - 1) * HC : U * HC], h0_src)
                    nc.vector.tensor_copy(c, c0_src)
                gsb = pw.tile([128, 12], f32, tag="gsb", name=f"gsb_{s}")
                gso = pw.tile([128, 4], f32, tag="gso", name=f"gso_{s}")
                sif = pw.tile([128, 8], f32, tag="sif", name=f"sif_{s}")
                sio = pw.tile([128, 4], f32, tag="sio", name=f"sio_{s}")
                tng = pw.tile([128, 4], f32, tag="tng", name=f"tng_{s}")
                tt1 = pw.tile([128, 4], f32, tag="tt1", name=f"tt1_{s}")
                tt2 = pw.tile([128, 4], f32, tag="tt2", name=f"tt2_{s}")
                tnc = pw.tile([128, 4], f32, tag="tnc", name=f"tnc_{s}")
                for w in range(T // CH):
                    t0 = w * CH
                    xw = xpw_pool.tile([128, GC, CH], f32, tag="win", name=f"xw_{s}_{w}")
                    nc.sync.dma_start(
                        out=xw,
                        in_=xp_dram[:, :].rearrange("p (g t) -> p g t", g=GC)[
                            :, :, t0 : t0 + CH
                        ],
                    )
                    with tc.For_i(0, CH // U, hint_engines=(ET.PE,)) as iv:
                        for u in range(U):
                            hprev = hbuf[:, ((u - 1) % U) * HC : ((u - 1) % U) * HC + HC]
                            psi = pss_pool.tile([128, 12], f32, tag="psi", name=f"psi_{s}_{u}")
                            pso = pss_pool.tile([128, 4], f32, tag="pso", name=f"pso_{s}_{u}")
                            for mc in range(12):
                                for kc in range(HC):
                                    nc.tensor.matmul(
                                        psi[:, mc : mc + 1],
                                        W[:, kc * G + mc * 128 : kc * G + (mc + 1) * 128],
                                        hprev[:, kc : kc + 1],
                                        start=(kc == 0),
                                        stop=(kc == HC - 1),
                                    )
                            for mc in range(12, 16):
                                for kc in range(HC):
                                    nc.tensor.matmul(
                                        pso[:, mc - 12 : mc - 11],
                                        W[:, kc * G + mc * 128 : kc * G + (mc + 1) * 128],
                                        hprev[:, kc : kc + 1],
                                        start=(kc == 0),
                                        stop=(kc == HC - 1),
                                    )
                            # i/f/g chain (overlaps o-gate matmuls)
                            nc.vector.tensor_tensor(
                                out=gsb, in0=psi, in1=xw[:, 0:12, ds(U * iv + u, 1)], op=ALU.add
                            )
                            nc.scalar.activation(sif, gsb[:, 0:8], AF.Sigmoid)
                            nc.scalar.activation(tng, gsb[:, 8:12], AF.Tanh)
                            nc.vector.tensor_tensor(out=tt2, in0=sif[:, 0:4], in1=tng, op=ALU.mult)
                            nc.vector.tensor_tensor(out=tt1, in0=sif[:, 4:8], in1=c, op=ALU.mult)
                            nc.vector.tensor_tensor(out=c, in0=tt1, in1=tt2, op=ALU.add)
                            # o gate
                            nc.vector.tensor_tensor(
                                out=gso, in0=pso, in1=xw[:, 12:16, ds(U * iv + u, 1)], op=ALU.add
                            )
                            nc.scalar.activation(sio, gso, AF.Sigmoid)
                            nc.scalar.activation(tnc, c, AF.Tanh)
                            nc.vector.tensor_tensor(
                                out=hbuf[:, u * HC : u * HC + HC],
                                in0=sio,
                                in1=tnc,
                                op=ALU.mult,
                            )
                        # spill this iteration's U hidden states to history
                        nc.vector.tensor_copy(
                            Hs[:, ds(HC * (t0 + 1) + HC * U * iv, HC * U)], hbuf
                        )

            # ---- AllGather of an Hs buffer; returns peer tile (peer's order).
            # Core-symmetric: peer block = (rank0 + rank1) - own, computed in
            # f32 chunks (exact for fp8 values).
            def exchange_hs(Hs, tagsuffix):
                nc.sync.dma_start(out=hs_ag_in[:, :], in_=Hs)
                nc.gpsimd.collective_compute(
                    "AllGather", ALU.bypass,
                    ins=[hs_ag_in[:, :]], outs=[hs_ag_out[:, :]], replica_groups=RG,
                )
                peer = peer_pool.tile(
                    [128, HC * (T + 1)], f8, tag="peer", name=f"peer_{tagsuffix}"
                )
                CW = 1026  # 8 chunks cover HC*(T+1) = 8196 (last chunk 1014)
                for ci in range(8):
                    lo = ci * CW
                    hi = min(HC * (T + 1), lo + CW)
                    n = hi - lo
                    b0 = peer_pool.tile([128, CW], f8, tag="pb0", name=f"pb0_{tagsuffix}_{ci}")
                    b1 = peer_pool.tile([128, CW], f8, tag="pb1", name=f"pb1_{tagsuffix}_{ci}")
                    nc.sync.dma_start(out=b0[:, :n], in_=hs_ag_out[0:128, lo:hi])
                    nc.sync.dma_start(out=b1[:, :n], in_=hs_ag_out[128:256, lo:hi])
                    pf = peer_pool.tile([128, CW], f32, tag="pf", name=f"pf_{tagsuffix}_{ci}")
                    nc.vector.tensor_tensor(out=pf[:, :n], in0=b0[:, :n], in1=b1[:, :n], op=ALU.add)
                    nc.vector.tensor_tensor(out=pf[:, :n], in0=pf[:, :n], in1=Hs[:, lo:hi], op=ALU.subtract)
                    nc.vector.tensor_copy(peer[:, lo:hi], pf[:, :n])
                return peer

            # reversed-read AP into peer Hs outputs: own-time t in [t0, t0+n),
            # chunk kc -> peer col HC*(T - t) + kc, step -HC
            def peer_rev_ap(peer, kc, t0, n):
                return peer[:, :].rearrange("p (t c) -> p t c", c=HC)[
                    :, T - t0 : T - t0 - n : -1, kc
                ]

            # ---- ENC pipeline
            Hs_e0 = hs_pool.tile([128, HC * (T + 1)], f8, tag="Hs", name="Hs_enc0")
            c_e0 = pw.tile([128, HC], f32, name="c_enc0")
            scan("enc0", xp_a, Hs_e0, c_e0)

            peer_e0 = exchange_hs(Hs_e0, "enc")
            own1 = slab1_pool.tile([128, HC * G], f8, tag="slab1", name="w1o_enc")
            nc.sync.dma_start(out=own1, in_=wih1o_d["enc"][:, :])
            peer1 = slab2_pool.tile([128, HC * G], f8, tag="slab2", name="w1p_enc")
            nc.sync.dma_start(out=peer1, in_=wih1p_d["enc"][:, :])
            xp_matmul(
                xp_a,
                [
                    (own1, HC, lambda kc, t0, n: Hs_e0[:, :].rearrange(
                        "p (t c) -> p t c", c=HC)[:, t0 + 1 : t0 + 1 + n, kc]),
                    (peer1, HC, lambda kc, t0, n: peer_rev_ap(peer_e0, kc, t0, n)),
                ],
                bias["enc1"],
            )
            Hs_e1 = hs_pool.tile([128, HC * (T + 1)], f8, tag="Hs", name="Hs_enc1")
            c_e1 = pw.tile([128, HC], f32, name="c_enc1")
            scan("enc1", xp_a, Hs_e1, c_e1)

            # ---- finals AG + init-state matvecs
            fin = pw.tile([128, 16], f32, name="fin")
            nc.vector.tensor_copy(fin[:, 0:4], Hs_e0[:, HC * T : HC * T + 4])
            nc.vector.tensor_copy(fin[:, 4:8], Hs_e1[:, HC * T : HC * T + 4])
            nc.vector.tensor_copy(fin[:, 8:12], c_e0)
            nc.vector.tensor_copy(fin[:, 12:16], c_e1)
            nc.sync.dma_start(out=fin_ag_in[:, :], in_=fin)
            nc.gpsimd.collective_compute(
                "AllGather", ALU.bypass,
                ins=[fin_ag_in[:, :]], outs=[fin_ag_out[:, :]], replica_groups=RG,
            )
            enc_all = pw.tile([128, 32], f32, name="enc_all")
            nc.sync.dma_start(out=enc_all[:, 0:16], in_=fin_ag_out[0:128, :])
            nc.sync.dma_start(out=enc_all[:, 16:32], in_=fin_ag_out[128:256, :])

            e2hb = pw.tile([128, 8], f32, name="e2hb")
            nc.sync.dma_start(out=e2hb, in_=e2hb_d[:, :])
            e2cb = pw.tile([128, 8], f32, name="e2cb")
            nc.sync.dma_start(out=e2cb, in_=e2cb_d[:, :])
            # rhs columns in AG order: h cols = enc_all [0:8] + [16:24];
            # c cols = [8:16] + [24:32]. enc_all must be bf16 for matmul.
            enc_all_bf = pw.tile([128, 32], bf16, name="enc_all_bf")
            nc.vector.tensor_copy(enc_all_bf, enc_all)
            hcols = list(range(0, 8)) + list(range(16, 24))
            ccols = list(range(8, 16)) + list(range(24, 32))
            init_h = pw.tile([128, 8], f32, name="init_h")
            init_c = pw.tile([128, 8], f32, name="init_c")
            for (wd, cols, bt, out_t) in (
                (e2hT_d, hcols, e2hb, init_h),
                (e2cT_d, ccols, e2cb, init_c),
            ):
                wr = wd[:, :].rearrange("p (k m) -> p k m", k=GC)
                ps = psx_pool.tile([128, 8], f32, tag="psx", name=f"ps_init_{out_t.name}")
                for m in range(8):
                    eww = xpw_pool.tile(
                        [128, GC, 128], bf16, tag="wwin", name=f"e2w_{out_t.name}_{m}"
                    )
                    nc.sync.dma_start(out=eww, in_=wr[:, :, m * 128 : (m + 1) * 128])
                    for kc in range(GC):
                        nc.tensor.matmul(
                            ps[:, m : m + 1],
                            eww[:, kc, :],
                            enc_all_bf[:, cols[kc] : cols[kc] + 1],
                            start=(kc == 0),
                            stop=(kc == GC - 1),
                        )
                nc.vector.tensor_tensor(out=out_t, in0=ps, in1=bt, op=ALU.add)

            # ---- DEC pipeline
            Hs_d0 = hs_pool.tile([128, HC * (T + 1)], f8, tag="Hs", name="Hs_dec0")
            c_d0 = pw.tile([128, HC], f32, name="c_dec0")
            scan("dec0", xp_b, Hs_d0, c_d0, init_h[:, 0:4], init_c[:, 0:4])

            peer_d0 = exchange_hs(Hs_d0, "dec")
            own1d = slab1_pool.tile([128, HC * G], f8, tag="slab1", name="w1o_dec")
            nc.sync.dma_start(out=own1d, in_=wih1o_d["dec"][:, :])
            peer1d = slab2_pool.tile([128, HC * G], f8, tag="slab2", name="w1p_dec")
            nc.sync.dma_start(out=peer1d, in_=wih1p_d["dec"][:, :])
            xp_matmul(
                xp_a,
                [
                    (own1d, HC, lambda kc, t0, n: Hs_d0[:, :].rearrange(
                        "p (t c) -> p t c", c=HC)[:, t0 + 1 : t0 + 1 + n, kc]),
                    (peer1d, HC, lambda kc, t0, n: peer_rev_ap(peer_d0, kc, t0, n)),
                ],
                bias["dec1"],
            )
            Hs_d1 = hs_pool.tile([128, HC * (T + 1)], f8, tag="Hs", name="Hs_dec1")
            c_d1 = pw.tile([128, HC], f32, name="c_dec1")
            scan("dec1", xp_a, Hs_d1, c_d1, init_h[:, 4:8], init_c[:, 4:8])

            # ---- feats: AllGather dec-L1 outputs; each core computes the
            # full feats identically (rank0 block = fwd dir ascending, rank1
            # block = bwd dir, read time-reversed).
            nc.sync.dma_start(out=hs_ag_in[:, :], in_=Hs_d1)
            nc.gpsimd.collective_compute(
                "AllGather", ALU.bypass,
                ins=[hs_ag_in[:, :]], outs=[hs_ag_out[:, :]], replica_groups=RG,
            )
            r0b = peer_pool.tile([128, HC * (T + 1)], f8, tag="peer", name="d1_r0")
            nc.sync.dma_start(out=r0b, in_=hs_ag_out[0:128, :])
            r1b = peer_pool.tile([128, HC * (T + 1)], f8, tag="peerb", name="d1_r1")
            nc.sync.dma_start(out=r1b, in_=hs_ag_out[128:256, :])
            h2tT0 = pw.tile([128, HC * K], f8, name="h2tT0")
            nc.sync.dma_start(out=h2tT0, in_=h2tT_r0_d[:, :])
            h2tT1 = pw.tile([128, HC * K], f8, name="h2tT1")
            nc.sync.dma_start(out=h2tT1, in_=h2tT_r1_d[:, :])
            feats = pw.tile([K, T], f32, name="feats")
            NT = 512
            r0r = r0b[:, :].rearrange("p (t c) -> p t c", c=HC)
            r1r = r1b[:, :].rearrange("p (t c) -> p t c", c=HC)
            for tb in range(T // NT):
                t0 = tb * NT
                ps = psx_pool.tile([K, NT], f32, tag="psx", name=f"psf_{tb}")
                for kc in range(HC):
                    nc.tensor.matmul(
                        ps, h2tT0[:, kc * K : (kc + 1) * K],
                        r0r[:, t0 + 1 : t0 + 1 + NT, kc],
                        start=(kc == 0), stop=False,
                    )
                for kc in range(HC):
                    nc.tensor.matmul(
                        ps, h2tT1[:, kc * K : (kc + 1) * K],
                        r1r[:, T - t0 : T - t0 - NT : -1, kc],
                        start=False, stop=(kc == HC - 1),
                    )
                nc.vector.tensor_copy(feats[:, t0 : t0 + NT], ps)
            h2tb = pw.tile([K, 1], f32, name="h2tb")
            nc.sync.dma_start(out=h2tb, in_=h2tb_d[:, :])
            nc.vector.tensor_scalar(
                out=feats, in0=feats, scalar1=h2tb, scalar2=None, op0=ALU.add
            )
            nc.sync.dma_start(out=feats_out[:, :], in_=feats)

            # ---- CRF forward (linear domain)
            expF = pw.tile([K, T], f32, name="expF")
            nc.scalar.activation(expF, feats, AF.Exp)
            transT_sb = pw.tile([K, K], f32, name="transT_sb")
            nc.sync.dma_start(out=transT_sb, in_=transT_d[:, :])
            PexpT = pw.tile([K, K], f32, name="PexpT")
            nc.scalar.activation(PexpT, transT_sb, AF.Exp)
            transEnd_sb = pw.tile([K, 1], f32, name="transEnd_sb")
            nc.sync.dma_start(out=transEnd_sb, in_=transEnd_d[:, :])
            expTE = pw.tile([K, 1], f32, name="expTE")
            nc.scalar.activation(expTE, transEnd_sb, AF.Exp)
            alpha = pw.tile([K, 1], f32, name="alpha")
            nc.sync.dma_start(out=alpha, in_=alpha0_d[:, :])
            ones48 = pw.tile([K, K], f32, name="ones48")
            nc.vector.memset(ones48, 1.0)
            lnS_sb = pw.tile([1, T], f32, name="lnS_sb")
            ut = pw.tile([K, 1], f32, name="ut")
            rs = pw.tile([K, 1], f32, name="rs")

            with tc.For_i(0, T // UCRF) as iv:
                for u in range(UCRF):
                    psA = psm_pool.tile([K, 1], f32, tag="psA", name=f"psA_{u}")
                    nc.tensor.matmul(psA, PexpT, alpha, start=True, stop=True)
                    nc.vector.tensor_tensor(
                        out=ut, in0=psA, in1=expF[:, ds(UCRF * iv + u, 1)], op=ALU.mult
                    )
                    psS = psm_pool.tile([K, 1], f32, tag="psA", name=f"psS_{u}")
                    nc.tensor.matmul(psS, ones48, ut, start=True, stop=True)
                    nc.scalar.activation(lnS_sb[:, ds(UCRF * iv + u, 1)], psS[0:1, :], AF.Ln)
                    nc.vector.reciprocal(rs, psS)
                    nc.vector.tensor_tensor(out=alpha, in0=ut, in1=rs, op=ALU.mult)
            psZ = psm_pool.tile([1, 1], f32, tag="psA", name="psZ")
            nc.tensor.matmul(psZ, alpha, expTE, start=True, stop=True)
            zf = pw.tile([1, 1], f32, name="zf")
            nc.scalar.activation(zf, psZ, AF.Ln)
            nc.sync.dma_start(out=zfin_out[:, :], in_=zf)
            nc.sync.dma_start(out=lnS_out[:, :], in_=lnS_sb)
    nc.compile()
    return nc


# ----------------------------------------------------------------------------
# entry point
# ----------------------------------------------------------------------------

def _postprocess(r0, inputs):
    feats = r0["feats"].astype(np.float64)  # [K, T]
    lnS = r0["lnS"].astype(np.float64)[0]
    zfin = float(r0["zfin"][0, 0])
    Z = float(lnS.sum() + zfin)

    tags = np.asarray(inputs["tags"]).astype(np.int64)
    trans = np.asarray(inputs["transitions"]).astype(np.float64)
    ext = np.concatenate([[START_IDX], tags])
    score = trans[ext[1:], ext[:-1]].sum() + feats[tags, np.arange(T)].sum()
    score += trans[END_IDX, tags[-1]]
    return np.float32(Z - score)


def kernel(**inputs) -> np.ndarray:
    if "nc" not in _CACHE:
        _CACHE["nc"] = build()
    nc = _CACHE["nc"]
    in_maps = [_prep_core(inputs, 0), _prep_core(inputs, 1)]
    res = run_bass_kernel_spmd(nc, in_maps, [0, 1])
    return _postprocess(res.results[0], inputs)


# revision 3
# speedup vs baseline: 6.9314x; 6.9314x over previous
"""BiLSTM Enc-Dec + CRF NLL loss on 2 Trainium2 cores (SPMD, fwd/bwd split).

Strategy
--------
Batch=1 sequence, T=2048. The four BiLSTM scans (enc L0 -> enc L1 -> dec L0
-> dec L1) are inherently sequential in time; within each layer the forward
and backward direction are independent. Core 0 runs all forward-direction
scans, core 1 runs all backward-direction scans, with one identical
(symmetric) SPMD program. Direction asymmetry is absorbed into per-core
*data*: core 1 receives the embedding sequence time-reversed so its
"forward" scan IS the backward scan; cross-core exchanges (layer outputs,
final states) use AllGather on internal DRAM bounce buffers.

Input projections x @ W_ih^T for a whole layer are big parallel matmuls
computed once per stage into DRAM (fp32), streamed into SBUF in windows
during the scan. The recurrent matvec h @ W_hh^T runs on the tensor engine
as 64 [128x128] fp8e4m3 weight-stationary matmuls per step (fp8 halves the
LDWEIGHTS time vs bf16 under FWL, which dominates at moving-dim 1). Gates
stay in the natural PyTorch order [i, f, g, o]; the i/f/g gate tiles issue
first so the cell-state update chain overlaps the o-gate matmuls. h is kept
in a small statically-addressed rotating buffer (hbuf) so the matmul moving
operands have compile-time addresses; one vector copy per unrolled loop
iteration spills hbuf into the full history Hs (fp8), which later feeds the
next layer's input projection, the cross-core exchange and the feats matmul.

The CRF forward pass runs in the linear domain: alpha' = exp(trans) @ alpha
(a single stationary 48x48 matmul per step) times exp(feats_t), renormalized
each step by its sum; log of the normalizer is accumulated on the host in
float64. The CRF score term is computed on the host from device feats.
"""

import sys

sys.path.insert(0, "/opt/trn_rl_repo")

import numpy as np
import ml_dtypes

import concourse.bacc as bacc
import concourse.mybir as mybir
from concourse.bass import ds
from concourse.tile import TileContext
from concourse.bass_utils import run_bass_kernel_spmd

# problem dims (hardcoded per spec)
T = 2048
ELMO = 1024
H = 512
POS = 64
K = 48
S = 50
L = 2
NEG = -10000.0
START_IDX, END_IDX = 0, 1

Din0 = ELMO + POS  # 1088
K0C = 9  # ceil(1088/128) k-tiles for layer-0 input (padded to 1152)
HC = 4  # h chunks of 128
G = 4 * H  # 2048 gates
GC = 16  # gate chunks of 128
U = 16  # scan steps unrolled per hardware-loop iteration
CH = 256  # scan steps per xp SBUF window
UCRF = 16

bf16 = mybir.dt.bfloat16
f8 = mybir.dt.float8e4
f32 = mybir.dt.float32
AF = mybir.ActivationFunctionType
ALU = mybir.AluOpType
ET = mybir.EngineType

np_f8 = ml_dtypes.float8_e4m3
np_bf16 = ml_dtypes.bfloat16

_CACHE = {}


# ----------------------------------------------------------------------------
# host-side weight preparation
# ----------------------------------------------------------------------------

def _tile_kT(wT, nk):
    """[Ktot, M] -> [128, nk*M] with col kc*M + m = wT[kc*128 + p, m]."""
    Ktot, M = wT.shape
    assert Ktot == nk * 128
    return np.ascontiguousarray(wT.reshape(nk, 128, M).transpose(1, 0, 2).reshape(128, nk * M))


def _prep_core(inputs, d):
    """Build the per-core input map for direction d (0=fwd core, 1=bwd core)."""
    f = np.float32
    ins = {}
    sentence = inputs["sentence"].astype(f)
    pos_emb = inputs["pos_emb"].astype(f)
    speech = inputs["speech_tags"].astype(np.int64)
    embeds = np.concatenate([sentence, pos_emb[speech]], axis=1)  # (T, 1088)
    if d == 1:
        embeds = embeds[::-1]
    embT = np.zeros((K0C * 128, T), f)
    embT[:Din0] = embeds.T
    ins["embT"] = _tile_kT(embT, K0C).astype(np_bf16)

    for model in ("enc", "dec"):
        for layer in (0, 1):
            whh = inputs[f"{model}_w_hh{layer}"][d].astype(f)  # (2048, 512)
            ins[f"whhT_{model}{layer}"] = _tile_kT(
                np.ascontiguousarray(whh.T), HC
            ).astype(np_f8)
            b = (inputs[f"{model}_b_ih{layer}"][d] + inputs[f"{model}_b_hh{layer}"][d]).astype(f)
            ins[f"bias_{model}{layer}"] = np.ascontiguousarray(
                b.reshape(GC, 128).T
            ).astype(f)  # [128,16] col mc
        wih0 = inputs[f"{model}_w_ih0"][d].astype(f)  # (2048, 1088)
        w0T = np.zeros((K0C * 128, G), f)
        w0T[:Din0] = wih0.T
        ins[f"wih0T_{model}"] = _tile_kT(w0T, K0C).astype(np_bf16)
        wih1 = inputs[f"{model}_w_ih1"][d].astype(f)  # (2048, 1024)
        own = wih1[:, d * H : (d + 1) * H]
        peer = wih1[:, (1 - d) * H : (2 - d) * H]
        ins[f"wih1T_own_{model}"] = _tile_kT(np.ascontiguousarray(own.T), HC).astype(np_f8)
        ins[f"wih1T_peer_{model}"] = _tile_kT(np.ascontiguousarray(peer.T), HC).astype(np_f8)

    # e2h/e2c: rows = own dec init states, cols permuted to AllGather order.
    # AG order of the 2048-dim enc state: [c0_l0, c0_l1, c1_l0, c1_l1]
    # (c0 = fwd dir, c1 = bwd dir); PyTorch flat order is [l0f, l0b, l1f, l1b].
    col_perm = np.concatenate(
        [
            np.arange(0, H),  # l0f
            np.arange(2 * H, 3 * H),  # l1f
            np.arange(H, 2 * H),  # l0b
            np.arange(3 * H, 4 * H),  # l1b
        ]
    )
    # own dec-init rows: init_h.reshape(2L, H)[j] is state for scan order
    # [dl0_f, dl0_b, dl1_f, dl1_b]; core d needs rows for [dl0 dir d, dl1 dir d]
    row_sel = np.concatenate([np.arange(d * H, (d + 1) * H), np.arange((2 + d) * H, (3 + d) * H)])
    for nm in ("e2h", "e2c"):
        w = inputs[f"{nm}_w"].astype(f)[row_sel][:, col_perm]  # (1024, 2048)
        ins[f"{nm}T"] = _tile_kT(np.ascontiguousarray(w.T), GC).astype(np_bf16)
        b = inputs[f"{nm}_b"].astype(f)[row_sel]  # (1024,)
        ins[f"{nm}_b"] = np.ascontiguousarray(b.reshape(8, 128).T).astype(f)  # [128, 8]

    # feats weights: rank0 half multiplies fwd-core outputs, rank1 half the
    # bwd-core outputs (identical on both cores; feats computed redundantly)
    h2t = inputs["h2t_w"].astype(f)
    ins["h2tT_r0"] = _tile_kT(np.ascontiguousarray(h2t[:, 0:H].T), HC).astype(np_f8)
    ins["h2tT_r1"] = _tile_kT(np.ascontiguousarray(h2t[:, H:].T), HC).astype(np_f8)
    ins["h2t_b"] = inputs["h2t_b"].astype(f).reshape(K, 1)

    trans = inputs["transitions"].astype(f)
    ins["transT"] = np.ascontiguousarray(trans.T)
    ins["transEnd"] = np.ascontiguousarray(trans[END_IDX].reshape(K, 1))
    a0 = np.full((K, 1), 0.0, f)
    a0[:, 0] = 0.0
    a0[START_IDX, 0] = 1.0
    ins["alpha0"] = a0
    return ins


# ----------------------------------------------------------------------------
# device program
# ----------------------------------------------------------------------------

def build():
    nc = bacc.Bacc("TRN2", target_bir_lowering=False, num_devices=2)

    def din(name, shape, dt=bf16):
        return nc.dram_tensor(name, shape, dt, kind="ExternalInput")

    embT_d = din("embT", [128, K0C * T])
    whh_d = {s: din(f"whhT_{s}", [128, HC * G], f8) for s in ("enc0", "enc1", "dec0", "dec1")}
    bias_d = {s: din(f"bias_{s}", [128, GC], f32) for s in ("enc0", "enc1", "dec0", "dec1")}
    wih0_d = {m: din(f"wih0T_{m}", [128, K0C * G]) for m in ("enc", "dec")}
    wih1o_d = {m: din(f"wih1T_own_{m}", [128, HC * G], f8) for m in ("enc", "dec")}
    wih1p_d = {m: din(f"wih1T_peer_{m}", [128, HC * G], f8) for m in ("enc", "dec")}
    e2hT_d = din("e2hT", [128, GC * 1024])
    e2cT_d = din("e2cT", [128, GC * 1024])
    e2hb_d = din("e2h_b", [128, 8], f32)
    e2cb_d = din("e2c_b", [128, 8], f32)
    h2tT_r0_d = din("h2tT_r0", [128, HC * K], f8)
    h2tT_r1_d = din("h2tT_r1", [128, HC * K], f8)
    h2tb_d = din("h2t_b", [K, 1], f32)
    transT_d = din("transT", [K, K], f32)
    transEnd_d = din("transEnd", [K, 1], f32)
    alpha0_d = din("alpha0", [K, 1], f32)

    feats_out = nc.dram_tensor("feats", [K, T], f32, kind="ExternalOutput")
    lnS_out = nc.dram_tensor("lnS", [1, T], f32, kind="ExternalOutput")
    zfin_out = nc.dram_tensor("zfin", [1, 1], f32, kind="ExternalOutput")

    # internal DRAM
    xp_a = nc.dram_tensor("xp_a", [128, GC * T], f32)  # enc0 / enc1 / dec1
    xp_b = nc.dram_tensor("xp_b", [128, GC * T], f32)  # dec0
    hs_ag_in = nc.dram_tensor("hs_ag_in", [128, HC * (T + 1)], f8)
    hs_ag_out = nc.dram_tensor("hs_ag_out", [256, HC * (T + 1)], f8)
    fin_ag_in = nc.dram_tensor("fin_ag_in", [128, 16], f32)
    fin_ag_out = nc.dram_tensor("fin_ag_out", [256, 16], f32)

    RG = [[0, 1]]

    with TileContext(nc) as tc:
        with (
            tc.tile_pool(name="pw", bufs=1) as pw,  # persistent weights/state
            tc.tile_pool(name="slab1", bufs=1) as slab1_pool,  # whh / wih1 own
            tc.tile_pool(name="slab2", bufs=1) as slab2_pool,  # wih1 peer
            tc.tile_pool(name="hs", bufs=2) as hs_pool,
            tc.tile_pool(name="peer", bufs=1) as peer_pool,
            tc.tile_pool(name="xpw", bufs=2) as xpw_pool,
            tc.tile_pool(name="psx", bufs=2, space="PSUM") as psx_pool,  # xp matmuls
            tc.tile_pool(name="pss", bufs=2, space="PSUM") as pss_pool,  # scan
            tc.tile_pool(name="psm", bufs=2, space="PSUM") as psm_pool,  # crf
        ):
            # ---- persistent loads
            bias = {}
            for s in ("enc0", "enc1", "dec0", "dec1"):
                bias[s] = pw.tile([128, GC], f32, name=f"bias_{s}")
                nc.sync.dma_start(out=bias[s], in_=bias_d[s][:, :])

            # ---- xp matmul helper: out_dram[:, mc*T + t] over given k-slabs
            def xp_matmul(out_dram, slabs, bias_tile):
                """slabs: list of (sbuf_slab, nk, rhs_fn) triples contracting
                consecutive k-ranges; rhs_fn(kc, t0, n) -> AP [128, n] moving."""
                NT = 512
                for tb in range(T // NT):
                    t0 = tb * NT
                    for mc in range(GC):
                        ps = psx_pool.tile([128, NT], f32, tag="psx", name=f"psx_{tb}_{mc}")
                        first = True
                        for slab, nk, rhs_fn in slabs:
                            for kc in range(nk):
                                nc.tensor.matmul(
                                    ps,
                                    slab[:, kc * G + mc * 128 : kc * G + (mc + 1) * 128],
                                    rhs_fn(kc, t0, NT),
                                    start=first,
                                    stop=(slab is slabs[-1][0]) and kc == nk - 1,
                                )
                                first = False
                        st = xpw_pool.tile([128, NT], f32, tag="xstage", name=f"xst_{tb}_{mc}")
                        nc.vector.tensor_scalar(
                            out=st, in0=ps, scalar1=bias_tile[:, mc : mc + 1],
                            scalar2=None, op0=ALU.add,
                        )
                        nc.sync.dma_start(
                            out=out_dram[:, mc * T + t0 : mc * T + t0 + NT], in_=st
                        )

            # ---- P0: layer-0 xp for enc and dec (embT and wih0 streamed
            # in windows; weight window per (tb, mc): [128, K0C, 128])
            embr = embT_d[:, :].rearrange("p (k t) -> p k t", k=K0C)
            NT = 512
            for model, out_dram in (("enc", xp_a), ("dec", xp_b)):
                w0r = wih0_d[model][:, :].rearrange("p (k m) -> p k m", k=K0C)
                for tb in range(T // NT):
                    t0 = tb * NT
                    ew = xpw_pool.tile([128, K0C, NT], bf16, tag="win", name=f"ew_{model}_{tb}")
                    nc.sync.dma_start(out=ew, in_=embr[:, :, t0 : t0 + NT])
                    for mc in range(GC):
                        ww = xpw_pool.tile(
                            [128, K0C, 128], bf16, tag="wwin", name=f"ww_{model}_{tb}_{mc}"
                        )
                        nc.sync.dma_start(
                            out=ww, in_=w0r[:, :, mc * 128 : (mc + 1) * 128]
                        )
                        ps = psx_pool.tile([128, NT], f32, tag="psx", name=f"psx0_{model}_{tb}_{mc}")
                        for kc in range(K0C):
                            nc.tensor.matmul(
                                ps, ww[:, kc, :], ew[:, kc, :],
                                start=(kc == 0), stop=(kc == K0C - 1),
                            )
                        st = xpw_pool.tile([128, NT], f32, tag="xstage", name=f"x0_{model}_{tb}_{mc}")
                        nc.vector.tensor_scalar(
                            out=st, in0=ps, scalar1=bias[f"{model}0"][:, mc : mc + 1],
                            scalar2=None, op0=ALU.add,
                        )
                        nc.sync.dma_start(
                            out=out_dram[:, mc * T + t0 : mc * T + t0 + NT], in_=st
                        )

            # ---- scan helper
            # gates in natural PyTorch order [i, f, g, o]; i/f/g tiles (mc
            # 0..11) issue first so the c-update overlaps the o-gate matmuls.
            def scan(s, xp_dram, Hs, c, h0_src=None, c0_src=None):
                """Run one LSTM direction scan. Hs: [128, HC*(T+1)] f8 tile;
                c: [128, HC] f32 tile. h0/c0 default zero."""
                W = slab1_pool.tile([128, HC * G], f8, tag="whh", name=f"whh_{s}")
                nc.sync.dma_start(out=W, in_=whh_d[s][:, :])
                hbuf = pw.tile([128, U * HC], f8, tag="hbuf", name=f"hbuf_{s}")
                if h0_src is None:
                    nc.vector.memset(Hs[:, 0:HC], 0.0)
                    nc.vector.memset(hbuf[:, (U - 1) * HC : U * HC], 0.0)
                    nc.vector.memset(c, 0.0)
                else:
                    nc.vector.tensor_copy(Hs[:, 0:HC], h0_src)
                    nc.vector.tensor_copy(hbuf[:, (U - 1) * HC : U * HC], h0_src)
                    nc.vector.tensor_copy(c, c0_src)
                gsb = pw.tile([128, 12], f32, tag="gsb", name=f"gsb_{s}")
                gso = pw.tile([128, 4], f32, tag="gso", name=f"gso_{s}")
                sif = pw.tile([128, 8], f32, tag="sif", name=f"sif_{s}")
                sio = pw.tile([128, 4], f32, tag="sio", name=f"sio_{s}")
                tng = pw.tile([128, 4], f32, tag="tng", name=f"tng_{s}")
                tt1 = pw.tile([128, 4], f32, tag="tt1", name=f"tt1_{s}")
                tt2 = pw.tile([128, 4], f32, tag="tt2", name=f"tt2_{s}")
                tnc = pw.tile([128, 4], f32, tag="tnc", name=f"tnc_{s}")
                for w in range(T // CH):
                    t0 = w * CH
                    xw = xpw_pool.tile([128, GC, CH], f32, tag="win", name=f"xw_{s}_{w}")
                    nc.sync.dma_start(
                        out=xw,
                        in_=xp_dram[:, :].rearrange("p (g t) -> p g t", g=GC)[
                            :, :, t0 : t0 + CH
                        ],
                    )
                    with tc.For_i(0, CH // U, hint_engines=(ET.PE,)) as iv:
                        for u in range(U):
                            hprev = hbuf[:, ((u - 1) % U) * HC : ((u - 1) % U) * HC + HC]
                            psi = pss_pool.tile([128, 12], f32, tag="psi", name=f"psi_{s}_{u}")
                            pso = pss_pool.tile([128, 4], f32, tag="pso", name=f"pso_{s}_{u}")
                            for mc in range(12):
                                for kc in range(HC):
                                    nc.tensor.matmul(
                                        psi[:, mc : mc + 1],
                                        W[:, kc * G + mc * 128 : kc * G + (mc + 1) * 128],
                                        hprev[:, kc : kc + 1],
                                        start=(kc == 0),
                                        stop=(kc == HC - 1),
                                    )
                            for mc in range(12, 16):
                                for kc in range(HC):
                                    nc.tensor.matmul(
                                        pso[:, mc - 12 : mc - 11],
                                        W[:, kc * G + mc * 128 : kc * G + (mc + 1) * 128],
                                        hprev[:, kc : kc + 1],
                                        start=(kc == 0),
                                        stop=(kc == HC - 1),
                                    )
                            # i/f/g chain (overlaps o-gate matmuls)
                            nc.vector.tensor_tensor(
                                out=gsb, in0=psi, in1=xw[:, 0:12, ds(U * iv + u, 1)], op=ALU.add
                            )
                            nc.scalar.activation(sif, gsb[:, 0:8], AF.Sigmoid)
                            nc.scalar.activation(tng, gsb[:, 8:12], AF.Tanh)
                            nc.vector.tensor_tensor(out=tt2, in0=sif[:, 0:4], in1=tng, op=ALU.mult)
                            nc.vector.tensor_tensor(out=tt1, in0=sif[:, 4:8], in1=c, op=ALU.mult)
                            nc.vector.tensor_tensor(out=c, in0=tt1, in1=tt2, op=ALU.add)
                            # o gate
                            nc.vector.tensor_tensor(
                                out=gso, in0=pso, in1=xw[:, 12:16, ds(U * iv + u, 1)], op=ALU.add
                            )
                            nc.scalar.activation(sio, gso, AF.Sigmoid)
                            nc.scalar.activation(tnc, c, AF.Tanh)
                            nc.vector.tensor_tensor(
                                out=hbuf[:, u * HC : u * HC + HC],
                                in0=sio,
                                in1=tnc,
                                op=ALU.mult,
                            )
                        # spill this iteration's U hidden states to history
                        nc.vector.tensor_copy(
                            Hs[:, ds(HC * (t0 + 1) + HC * U * iv, HC * U)], hbuf
                        )

            # ---- AllGather of an Hs buffer; returns peer tile (peer's order).
            # Core-symmetric: peer block = (rank0 + rank1) - own, computed in
            # f32 chunks (exact for fp8 values).
            def exchange_hs(Hs, tagsuffix):
                nc.sync.dma_start(out=hs_ag_in[:, :], in_=Hs)
                nc.gpsimd.collective_compute(
                    "AllGather", ALU.bypass,
                    ins=[hs_ag_in[:, :]], outs=[hs_ag_out[:, :]], replica_groups=RG,
                )
                peer = peer_pool.tile(
                    [128, HC * (T + 1)], f8, tag="peer", name=f"peer_{tagsuffix}"
                )
                CW = 1026  # 8 chunks cover HC*(T+1) = 8196 (last chunk 1014)
                for ci in range(8):
                    lo = ci * CW
                    hi = min(HC * (T + 1), lo + CW)
                    n = hi - lo
                    b0 = peer_pool.tile([128, CW], f8, tag="pb0", name=f"pb0_{tagsuffix}_{ci}")
                    b1 = peer_pool.tile([128, CW], f8, tag="pb1", name=f"pb1_{tagsuffix}_{ci}")
                    nc.sync.dma_start(out=b0[:, :n], in_=hs_ag_out[0:128, lo:hi])
                    nc.sync.dma_start(out=b1[:, :n], in_=hs_ag_out[128:256, lo:hi])
                    pf = peer_pool.tile([128, CW], f32, tag="pf", name=f"pf_{tagsuffix}_{ci}")
                    nc.vector.tensor_tensor(out=pf[:, :n], in0=b0[:, :n], in1=b1[:, :n], op=ALU.add)
                    nc.vector.tensor_tensor(out=pf[:, :n], in0=pf[:, :n], in1=Hs[:, lo:hi], op=ALU.subtract)
                    nc.vector.tensor_copy(peer[:, lo:hi], pf[:, :n])
                return peer

            # reversed-read AP into peer Hs outputs: own-time t in [t0, t0+n),
            # chunk kc -> peer col HC*(T - t) + kc, step -HC
            def peer_rev_ap(peer, kc, t0, n):
                return peer[:, :].rearrange("p (t c) -> p t c", c=HC)[
                    :, T - t0 : T - t0 - n : -1, kc
                ]

            # ---- ENC pipeline
            Hs_e0 = hs_pool.tile([128, HC * (T + 1)], f8, tag="Hs", name="Hs_enc0")
            c_e0 = pw.tile([128, HC], f32, name="c_enc0")
            scan("enc0", xp_a, Hs_e0, c_e0)

            peer_e0 = exchange_hs(Hs_e0, "enc")
            own1 = slab1_pool.tile([128, HC * G], f8, tag="slab1", name="w1o_enc")
            nc.sync.dma_start(out=own1, in_=wih1o_d["enc"][:, :])
            peer1 = slab2_pool.tile([128, HC * G], f8, tag="slab2", name="w1p_enc")
            nc.sync.dma_start(out=peer1, in_=wih1p_d["enc"][:, :])
            xp_matmul(
                xp_a,
                [
                    (own1, HC, lambda kc, t0, n: Hs_e0[:, :].rearrange(
                        "p (t c) -> p t c", c=HC)[:, t0 + 1 : t0 + 1 + n, kc]),
                    (peer1, HC, lambda kc, t0, n: peer_rev_ap(peer_e0, kc, t0, n)),
                ],
                bias["enc1"],
            )
            Hs_e1 = hs_pool.tile([128, HC * (T + 1)], f8, tag="Hs", name="Hs_enc1")
            c_e1 = pw.tile([128, HC], f32, name="c_enc1")
            scan("enc1", xp_a, Hs_e1, c_e1)

            # ---- finals AG + init-state matvecs
            fin = pw.tile([128, 16], f32, name="fin")
            nc.vector.tensor_copy(fin[:, 0:4], Hs_e0[:, HC * T : HC * T + 4])
            nc.vector.tensor_copy(fin[:, 4:8], Hs_e1[:, HC * T : HC * T + 4])
            nc.vector.tensor_copy(fin[:, 8:12], c_e0)
            nc.vector.tensor_copy(fin[:, 12:16], c_e1)
            nc.sync.dma_start(out=fin_ag_in[:, :], in_=fin)
            nc.gpsimd.collective_compute(
                "AllGather", ALU.bypass,
                ins=[fin_ag_in[:, :]], outs=[fin_ag_out[:, :]], replica_groups=RG,
            )
            enc_all = pw.tile([128, 32], f32, name="enc_all")
            nc.sync.dma_start(out=enc_all[:, 0:16], in_=fin_ag_out[0:128, :])
            nc.sync.dma_start(out=enc_all[:, 16:32], in_=fin_ag_out[128:256, :])

            e2hb = pw.tile([128, 8], f32, name="e2hb")
            nc.sync.dma_start(out=e2hb, in_=e2hb_d[:, :])
            e2cb = pw.tile([128, 8], f32, name="e2cb")
            nc.sync.dma_start(out=e2cb, in_=e2cb_d[:, :])
            # rhs columns in AG order: h cols = enc_all [0:8] + [16:24];
            # c cols = [8:16] + [24:32]. enc_all must be bf16 for matmul.
            enc_all_bf = pw.tile([128, 32], bf16, name="enc_all_bf")
            nc.vector.tensor_copy(enc_all_bf, enc_all)
            hcols = list(range(0, 8)) + list(range(16, 24))
            ccols = list(range(8, 16)) + list(range(24, 32))
            init_h = pw.tile([128, 8], f32, name="init_h")
            init_c = pw.tile([128, 8], f32, name="init_c")
            for (wd, cols, bt, out_t) in (
                (e2hT_d, hcols, e2hb, init_h),
                (e2cT_d, ccols, e2cb, init_c),
            ):
                wr = wd[:, :].rearrange("p (k m) -> p k m", k=GC)
                ps = psx_pool.tile([128, 8], f32, tag="psx", name=f"ps_init_{out_t.name}")
                for m in range(8):
                    eww = xpw_pool.tile(
                        [128, GC, 128], bf16, tag="wwin", name=f"e2w_{out_t.name}_{m}"
                    )
                    nc.sync.dma_start(out=eww, in_=wr[:, :, m * 128 : (m + 1) * 128])
                    for kc in range(GC):
                        nc.tensor.matmul(
                            ps[:, m : m + 1],
                            eww[:, kc, :],
                            enc_all_bf[:, cols[kc] : cols[kc] + 1],
                            start=(kc == 0),
                            stop=(kc == GC - 1),
                        )
                nc.vector.tensor_tensor(out=out_t, in0=ps, in1=bt, op=ALU.add)

            # ---- DEC pipeline
            Hs_d0 = hs_pool.tile([128, HC * (T + 1)], f8, tag="Hs", name="Hs_dec0")
            c_d0 = pw.tile([128, HC], f32, name="c_dec0")
            scan("dec0", xp_b, Hs_d0, c_d0, init_h[:, 0:4], init_c[:, 0:4])

            peer_d0 = exchange_hs(Hs_d0, "dec")
            own1d = slab1_pool.tile([128, HC * G], f8, tag="slab1", name="w1o_dec")
            nc.sync.dma_start(out=own1d, in_=wih1o_d["dec"][:, :])
            peer1d = slab2_pool.tile([128, HC * G], f8, tag="slab2", name="w1p_dec")
            nc.sync.dma_start(out=peer1d, in_=wih1p_d["dec"][:, :])
            xp_matmul(
                xp_a,
                [
                    (own1d, HC, lambda kc, t0, n: Hs_d0[:, :].rearrange(
                        "p (t c) -> p t c", c=HC)[:, t0 + 1 : t0 + 1 + n, kc]),
                    (peer1d, HC, lambda kc, t0, n: peer_rev_ap(peer_d0, kc, t0, n)),
                ],
                bias["dec1"],
            )
            Hs_d1 = hs_pool.tile([128, HC * (T + 1)], f8, tag="Hs", name="Hs_dec1")
            c_d1 = pw.tile([128, HC], f32, name="c_dec1")
            scan("dec1", xp_a, Hs_d1, c_d1, init_h[:, 4:8], init_c[:, 4:8])

            # ---- feats: AllGather dec-L1 outputs; each core computes the
            # full feats identically (rank0 block = fwd dir ascending, rank1
            # block = bwd dir, read time-reversed).
            nc.sync.dma_start(out=hs_ag_in[:, :], in_=Hs_d1)
            nc.gpsimd.collective_compute(
                "AllGather", ALU.bypass,
                ins=[hs_ag_in[:, :]], outs=[hs_ag_out[:, :]], replica_groups=RG,
            )
            r0b = peer_pool.tile([128, HC * (T + 1)], f8, tag="peer", name="d1_r0")
            nc.sync.dma_start(out=r0b, in_=hs_ag_out[0:128, :])
            r1b = peer_pool.tile([128, HC * (T + 1)], f8, tag="peerb", name="d1_r1")
            nc.sync.dma_start(out=r1b, in_=hs_ag_out[128:256, :])
            h2tT0 = pw.tile([128, HC * K], f8, name="h2tT0")
            nc.sync.dma_start(out=h2tT0, in_=h2tT_r0_d[:, :])
            h2tT1 = pw.tile([128, HC * K], f8, name="h2tT1")
            nc.sync.dma_start(out=h2tT1, in_=h2tT_r1_d[:, :])
            feats = pw.tile([K, T], f32, name="feats")
            NT = 512
            r0r = r0b[:, :].rearrange("p (t c) -> p t c", c=HC)
            r1r = r1b[:, :].rearrange("p (t c) -> p t c", c=HC)
            for tb in range(T // NT):
                t0 = tb * NT
                ps = psx_pool.tile([K, NT], f32, tag="psx", name=f"psf_{tb}")
                for kc in range(HC):
                    nc.tensor.matmul(
                        ps, h2tT0[:, kc * K : (kc + 1) * K],
                        r0r[:, t0 + 1 : t0 + 1 + NT, kc],
                        start=(kc == 0), stop=False,
                    )
                for kc in range(HC):
                    nc.tensor.matmul(
                        ps, h2tT1[:, kc * K : (kc + 1) * K],
                        r1r[:, T - t0 : T - t0 - NT : -1, kc],
                        start=False, stop=(kc == HC - 1),
                    )
                nc.vector.tensor_copy(feats[:, t0 : t0 + NT], ps)
            h2tb = pw.tile([K, 1], f32, name="h2tb")
            nc.sync.dma_start(out=h2tb, in_=h2tb_d[:, :])
            nc.vector.tensor_scalar(
                out=feats, in0=feats, scalar1=h2tb, scalar2=None, op0=ALU.add
            )
            nc.sync.dma_start(out=feats_out[:, :], in_=feats)

            # ---- CRF forward (linear domain)
            expF = pw.tile([K, T], f32, name="expF")
            nc.scalar.activation(expF, feats, AF.Exp)
            transT_sb = pw.tile([K, K], f32, name="transT_sb")
            nc.sync.dma_start(out=transT_sb, in_=transT_d[:, :])
            PexpT = pw.tile([K, K], f32, name="PexpT")
            nc.scalar.activation(PexpT, transT_sb, AF.Exp)
            transEnd_sb = pw.tile([K, 1], f32, name="transEnd_sb")
            nc.sync.dma_start(out=transEnd_sb, in_=transEnd_d[:, :])
            expTE = pw.tile([K, 1], f32, name="expTE")
            nc.scalar.activation(expTE, transEnd_sb, AF.Exp)
            alpha = pw.tile([K, 1], f32, name="alpha")
            nc.sync.dma_start(out=alpha, in_=alpha0_d[:, :])
            ones48 = pw.tile([K, K], f32, name="ones48")
            nc.vector.memset(ones48, 1.0)
            lnS_sb = pw.tile([1, T], f32, name="lnS_sb")
            ut = pw.tile([K, 1], f32, name="ut")
            rs = pw.tile([K, 1], f32, name="rs")

            with tc.For_i(0, T // UCRF) as iv:
                for u in range(UCRF):
                    psA = psm_pool.tile([K, 1], f32, tag="psA", name=f"psA_{u}")
                    nc.tensor.matmul(psA, PexpT, alpha, start=True, stop=True)
                    nc.vector.tensor_tensor(
                        out=ut, in0=psA, in1=expF[:, ds(UCRF * iv + u, 1)], op=ALU.mult
                    )
                    psS = psm_pool.tile([K, 1], f32, tag="psA", name=f"psS_{u}")
                    nc.tensor.matmul(psS, ones48, ut, start=True, stop=True)
                    nc.scalar.activation(lnS_sb[:, ds(UCRF * iv + u, 1)], psS[0:1, :], AF.Ln)
                    nc.vector.reciprocal(rs, psS)
                    nc.vector.tensor_tensor(out=alpha, in0=ut, in1=rs, op=ALU.mult)
            psZ = psm_pool.tile([1, 1], f32, tag="psA", name="psZ")
            nc.tensor.matmul(psZ, alpha, expTE, start=True, stop=True)
            zf = pw.tile([1, 1], f32, name="zf")
            nc.scalar.activation(zf, psZ, AF.Ln)
            nc.sync.dma_start(out=zfin_out[:, :], in_=zf)
            nc.sync.dma_start(out=lnS_out[:, :], in_=lnS_sb)
    nc.compile()
    return nc


# ----------------------------------------------------------------------------
# entry point
# ----------------------------------------------------------------------------

def _postprocess(r0, inputs):
    feats = r0["feats"].astype(np.float64)  # [K, T]
    lnS = r0["lnS"].astype(np.float64)[0]
    zfin = float(r0["zfin"][0, 0])
    Z = float(lnS.sum() + zfin)

    tags = np.asarray(inputs["tags"]).astype(np.int64)
    trans = np.asarray(inputs["transitions"]).astype(np.float64)
    ext = np.concatenate([[START_IDX], tags])
    score = trans[ext[1:], ext[:-1]].sum() + feats[tags, np.arange(T)].sum()
    score += trans[END_IDX, tags[-1]]
    return np.float32(Z - score)


def kernel(**inputs) -> np.ndarray:
    if "nc" not in _CACHE:
        _CACHE["nc"] = build()
    nc = _CACHE["nc"]
    in_maps = [_prep_core(inputs, 0), _prep_core(inputs, 1)]
    res = run_bass_kernel_spmd(nc, in_maps, [0, 1])
    return _postprocess(res.results[0], inputs)
